# revision 12
# baseline (speedup 1.0000x reference)
"""Bidirectional Mamba block (nn_Block_bi_mamba) Trainium2 Bass kernel.

Sharding: 8 cores = (batch b in {0,1}) x (d_inner quarter dq in {0..3}).
Each core computes, for its batch and both scan directions, the full
in_proj+conv (folded into PE matmuls) and x_proj (contracts over all 512
channels), the selective scan for its own 128 channels, and the out_proj
partial product [256, L]. The host sums the 4 partials per batch and
adds the residual x. The d_inner axis is permuted per core so the core's
own channel block is always channel-tile 0, making the device program
identical across cores (SPMD) with only input data differing.

Device layout: d-major [128 chans, time]. The causal depthwise conv is
folded into the in_proj matmul (8 shifted PSUM-accumulated matmuls).
The SSM recurrence runs on the DVE tensor_tensor_scan instruction
(state = dA*state + dBu along the free dim); the scan cluster is all
fp16 (2-byte operands run the scan at ~2x the fp32 rate; fp16 keeps the
near-1.0 resolution of dA that bf16 lacks). B/C rows are broadcast
across partitions via DRAM-source stride-0 DMA. The per-(dir,chunk)
front-end (conv matmuls -> silu -> x_proj -> dt) is software-pipelined
one step ahead of the scan phase to hide its serial latency.

Self-contained: hardcodes all shapes; no sibling imports.
"""
import numpy as np
import ml_dtypes
from contextlib import ExitStack

import concourse.bacc as bacc
import concourse.bass as bass
import concourse.tile as tile
from concourse import mybir
from concourse.bass_utils import run_bass_kernel_spmd
from concourse.alu_op_type import AluOpType as CCE

bf = ml_dtypes.bfloat16
FP32 = mybir.dt.float32
BF16 = mybir.dt.bfloat16
FP16 = mybir.dt.float16

B, L = 2, 4096
LC = 2048
NCH = L // LC
NSUB = LC // 512
N = 16
AOP = mybir.AluOpType
AF = mybir.ActivationFunctionType


def _bcast_from_dram(nc, dst, row):
    """DMA-broadcast a [1, F] DRAM row across all partitions of dst."""
    rap = [list(x) for x in row.ap]
    src = bass.AP(tensor=row.tensor, offset=row.offset,
                  ap=[[0, dst.shape[0]], rap[1]])
    nc.sync.dma_start(out=dst, in_=src)


def build_program(tc, ins, outs):
    nc = tc.nc
    with ExitStack() as ctx:
        wp = ctx.enter_context(tc.tile_pool(name="wp", bufs=1))
        big = ctx.enter_context(tc.tile_pool(name="big", bufs=1))
        work = ctx.enter_context(tc.tile_pool(name="work", bufs=1))
        scanp = ctx.enter_context(tc.tile_pool(name="scanp", bufs=2))
        ps = ctx.enter_context(tc.tile_pool(name="ps", bufs=2, space="PSUM"))
        dramp = ctx.enter_context(tc.tile_pool(name="dramp", bufs=3,
                                               space="DRAM"))

        # ---- weights ----
        # wconst fp32 [128, 76]: An_f 0:16 | An_b 16:32 | (unused) |
        #   convb_f 64:68 | convb_b 68:72 | Dd_f 72 | Dd_b 73 |
        #   dtbias_f 74 | dtbias_b 75
        wconst = wp.tile([128, 76], FP32, tag="wconst")
        nc.sync.dma_start(out=wconst, in_=ins["wconst"])
        COL = {"An_f": 0, "An_b": 16, "cb_f": 64, "cb_b": 68, "Dd_f": 72,
               "Dd_b": 73, "dtb_f": 74, "dtb_b": 75}

        wconv = wp.tile([128, 16 * 512], BF16, tag="wconv")
        nc.sync.dma_start(out=wconv, in_=ins["wconvP"])
        wz = wp.tile([128, 256], BF16, tag="wz")
        nc.sync.dma_start(out=wz, in_=ins["wzP"])
        outw = wp.tile([128, 256], FP16, tag="outw")
        nc.sync.dma_start(out=outw, in_=ins["outWT"])
        xpro = wp.tile([128, 384], FP16, tag="xpro")
        nc.sync.dma_start(out=xpro, in_=ins["xprojP"])
        dtprojp = wp.tile([16, 256], FP16, tag="dtprojp")
        nc.sync.dma_start(out=dtprojp, in_=ins["dtprojp"])
        carry = wp.tile([128, 32], FP32, tag="carry")

        # ---- persistent buffers ----
        xtp = [big.tile([128, L + 6], BF16, tag=f"xtp{kt}", name=f"xtp{kt}")
               for kt in range(2)]
        for kt in range(2):
            nc.sync.dma_start(out=xtp[kt],
                              in_=ins["xT16p"][kt * 128:(kt + 1) * 128])
        zs_all = big.tile([128, L], FP16, tag="zs")
        y_ball = big.tile([128, L], FP16, tag="yball")

        # ---- phase Z (emitted as a function; interleaved below) ----
        def phase_z(c):
            cr = slice(c * LC, (c + 1) * LC)
            zsb = work.tile([128, LC], FP16, tag="zsb")
            for nsub in range(NSUB):
                pt = ps.tile([128, 512], FP32, tag="ps_z", bufs=1)
                for kt in range(2):
                    nc.tensor.matmul(
                        pt, wz[:, kt * 128:(kt + 1) * 128],
                        xtp[kt][:, 3 + c * LC + nsub * 512:
                                3 + c * LC + (nsub + 1) * 512],
                        start=(kt == 0), stop=(kt == 1))
                nc.scalar.copy(out=zsb[:, nsub * 512:(nsub + 1) * 512],
                               in_=pt)
            sgz = work.tile([128, LC], FP16, tag="sgz")
            nc.scalar.activation(out=sgz, in_=zsb, func=AF.Sigmoid,
                                 bias=0.0, scale=1.0)
            nc.vector.tensor_tensor(zs_all[:, cr], zsb, sgz, AOP.mult)

        combos = ([("b", c) for c in range(NCH - 1, -1, -1)]
                  + [("f", c) for c in range(NCH)])

        def front_end(d, c):
            """conv+silu -> xc; x_proj -> dbl + DRAM scratch; dt."""
            dcol = 0 if d == "f" else 1
            base = 0 if d == "f" else 3
            cb0 = COL[f"cb_{d}"]
            xc = [work.tile([128, LC], FP16, tag=f"xc{t}", name=f"xc{t}",
                            bufs=(2 if t == 0 else 1)) for t in range(4)]
            for mt in range(4):
                vcv = work.tile([128, LC], FP16, tag="vcv", bufs=2)
                for nsub in range(NSUB):
                    pt = ps.tile([128, 512], FP32, tag="ps_conv")
                    ns0 = c * LC + nsub * 512 + base
                    for idx, (k, kt) in enumerate(
                            (k, kt) for k in range(4) for kt in range(2)):
                        seg = ((dcol * 4 + k) * 2 + kt) * 512
                        nc.tensor.matmul(
                            pt, wconv[:, seg + mt * 128:seg + (mt + 1) * 128],
                            xtp[kt][:, ns0 + k:ns0 + k + 512],
                            start=(idx == 0), stop=(idx == 7))
                    # v = psum + conv_b via table-free Identity(+bias)
                    nc.scalar.activation(
                        out=vcv[:, nsub * 512:(nsub + 1) * 512], in_=pt,
                        func=AF.Identity,
                        bias=wconst[:, cb0 + mt:cb0 + mt + 1], scale=1.0)
                sgb = work.tile([128, LC], FP16, tag="sgb", bufs=2)
                nc.scalar.activation(out=sgb, in_=vcv, func=AF.Sigmoid,
                                     bias=0.0, scale=1.0)
                nc.vector.tensor_tensor(xc[mt], vcv, sgb, AOP.mult)

            dbl = work.tile([48, LC], FP16, tag="dbl", bufs=2)
            for nsub in range(NSUB):
                pj = ps.tile([48, 512], FP32, tag="ps_xp", bufs=1)
                for kt in range(4):
                    nc.tensor.matmul(
                        pj, xpro[:, kt * 96 + 48 * dcol:
                                 kt * 96 + 48 * (dcol + 1)],
                        xc[kt][:, nsub * 512:(nsub + 1) * 512],
                        start=(kt == 0), stop=(kt == 3))
                nc.scalar.copy(out=dbl[:, nsub * 512:(nsub + 1) * 512],
                               in_=pj)
            scratch = dramp.tile([32, LC], FP16, tag="bcdram")
            nc.sync.dma_start(out=scratch, in_=dbl[16:48, :])

            dt = work.tile([128, LC], FP16, tag="dt", bufs=2)
            for nsub in range(NSUB):
                pt = ps.tile([128, 512], FP32, tag="ps_dt")
                nc.tensor.matmul(
                    pt, dtprojp[:, dcol * 128:(dcol + 1) * 128],
                    dbl[0:16, nsub * 512:(nsub + 1) * 512],
                    start=True, stop=True)
                esub = work.tile([128, 512], FP16, tag="esub", bufs=2)
                nc.scalar.activation(
                    out=esub, in_=pt, func=AF.Exp,
                    bias=wconst[:, COL[f"dtb_{d}"]:COL[f"dtb_{d}"] + 1],
                    scale=1.0)
                nc.scalar.activation(
                    out=dt[:, nsub * 512:(nsub + 1) * 512], in_=esub,
                    func=AF.Ln, bias=1.0, scale=1.0)

            du = work.tile([128, LC], FP16, tag="du", bufs=2)
            nc.vector.tensor_tensor(du, dt, xc[0], AOP.mult)
            return {"xc0": xc[0], "dt": dt, "du": du, "scratch": scratch}

        def scan_phase(d, c, st, first):
            dcol = 0 if d == "f" else 1
            rev = (lambda ap: ap[:, ::-1]) if d == "b" else (lambda ap: ap)
            cr = slice(c * LC, (c + 1) * LC)
            dt, du, scratch, u = st["dt"], st["du"], st["scratch"], st["xc0"]

            yacc = scanp.tile([128, LC], FP16, tag="yacc", bufs=2)
            for n in range(N):
                dA = scanp.tile([128, LC], FP16, tag="dA", bufs=3)
                nc.scalar.activation(
                    out=dA, in_=dt, func=AF.Exp, bias=0.0,
                    scale=wconst[:, COL[f"An_{d}"] + n:COL[f"An_{d}"] + n + 1])
                bbc = scanp.tile([128, LC], FP16, tag="bbc", bufs=3)
                _bcast_from_dram(nc, bbc, scratch[n:n + 1, :])
                nc.vector.tensor_tensor(bbc, du, bbc, AOP.mult)
                h = scanp.tile([128, LC], FP16, tag="h", bufs=3)
                init = (0.0 if first
                        else carry[:, dcol * 16 + n:dcol * 16 + n + 1])
                nc.vector.tensor_tensor_scan(h, rev(dA), rev(bbc), init,
                                             AOP.mult, AOP.add)
                if first and NCH > 1:
                    nc.vector.tensor_copy(
                        out=carry[:, dcol * 16 + n:dcol * 16 + n + 1],
                        in_=h[:, LC - 1:LC])
                cbc = scanp.tile([128, LC], FP16, tag="cbc", bufs=3)
                _bcast_from_dram(nc, cbc, scratch[16 + n:17 + n, :])
                nc.vector.tensor_tensor(h, h, rev(cbc), AOP.mult)
                if n == 0:
                    nc.vector.tensor_copy(out=yacc, in_=h)
                else:
                    nc.vector.tensor_tensor(yacc, yacc, h, AOP.add)

            yfin = yacc
            Dsl = wconst[:, COL[f"Dd_{d}"]:COL[f"Dd_{d}"] + 1]

            if d == "b":
                nc.vector.scalar_tensor_tensor(
                    y_ball[:, cr], u, Dsl, yfin[:, ::-1], AOP.mult, AOP.add)
            else:
                yg = work.tile([128, LC], FP16, tag="ytmp", bufs=2)
                nc.vector.scalar_tensor_tensor(
                    yg, u, Dsl, yfin, AOP.mult, AOP.add)
                ysum = work.tile([128, LC], FP16, tag="ytmp", bufs=2)
                nc.vector.tensor_tensor(ysum, yg, y_ball[:, cr], AOP.add)
                ygated = work.tile([128, LC], FP16, tag="ytmp", bufs=2)
                nc.vector.tensor_tensor(ygated, ysum, zs_all[:, cr], AOP.mult)
                for mt in range(2):
                    osb = work.tile([128, LC], FP32, tag="osb")
                    for nsub in range(NSUB):
                        po = ps.tile([128, 512], FP32, tag="ps_out")
                        nc.tensor.matmul(
                            po, outw[:, mt * 128:(mt + 1) * 128],
                            ygated[:, nsub * 512:(nsub + 1) * 512],
                            start=True, stop=True)
                        nc.scalar.copy(
                            out=osb[:, nsub * 512:(nsub + 1) * 512], in_=po)
                    nc.sync.dma_start(
                        out=outs["attnT"][mt * 128:(mt + 1) * 128, cr],
                        in_=osb)

        # software pipeline: front_end one combo ahead of the scan phase;
        # phase-Z rides in the shadow of the first front-end
        states = {}
        states[0] = front_end(*combos[0])
        for c in range(NCH):
            phase_z(c)
        for j, (d, c) in enumerate(combos):
            if j + 1 < len(combos):
                states[j + 1] = front_end(*combos[j + 1])
            first = (j % NCH == 0)
            scan_phase(d, c, states.pop(j), first)


def build_nc():
    nc = bacc.Bacc("TRN2", target_bir_lowering=False, debug=False,
                   enable_asserts=False)
    ins = {}

    def inp(name, shape, dt):
        ins[name] = nc.dram_tensor(name, shape, dt,
                                   kind="ExternalInput").ap()

    inp("xT16p", [256, L + 6], BF16)
    inp("wconvP", [128, 16 * 512], BF16)
    inp("wzP", [128, 256], BF16)
    inp("outWT", [128, 256], FP16)
    inp("xprojP", [128, 384], FP16)
    inp("dtprojp", [16, 256], FP16)
    inp("wconst", [128, 76], FP32)
    outs = {"attnT": nc.dram_tensor("attnT", [256, L], FP32,
                                    kind="ExternalOutput").ap()}
    with tile.TileContext(nc) as tc:
        build_program(tc, ins, outs)
    nc.compile()
    return nc


def prep_core_inputs(inputs, b, dq):
    """Per-core input arrays; d_inner axis permuted so own block is first."""
    own = np.arange(dq * 128, (dq + 1) * 128)
    rest = np.array([i for i in range(512)
                     if not (dq * 128 <= i < (dq + 1) * 128)])
    perm = np.concatenate([own, rest])

    out = {}
    xT = inputs["x"][b].T.astype(np.float32)  # [256, L]
    xTp = np.zeros((256, L + 6), np.float32)
    xTp[:, 3:L + 3] = xT
    out["xT16p"] = xTp.astype(bf)

    w_inx = inputs["in_proj_w"][:512][perm].astype(np.float32)  # [512, 256]
    wconvP = np.zeros((128, 16 * 512), np.float32)
    for dcol, d in enumerate("fb"):
        cw = inputs[f"conv_w_{d}"][:, 0, :][perm].astype(np.float32)
        for k in range(4):
            tap = cw[:, k] if d == "f" else cw[:, 3 - k]
            WdkT = (tap[:, None] * w_inx).T     # [256, 512]
            for kt in range(2):
                seg = ((dcol * 4 + k) * 2 + kt) * 512
                wconvP[:, seg:seg + 512] = WdkT[kt * 128:(kt + 1) * 128]
    out["wconvP"] = wconvP.astype(bf)

    wz = inputs["in_proj_w"][512:1024][own].astype(np.float32)  # [128, 256]
    wzP = np.zeros((128, 256), np.float32)
    for kt in range(2):
        wzP[:, kt * 128:(kt + 1) * 128] = wz.T[kt * 128:(kt + 1) * 128]
    out["wzP"] = wzP.astype(bf)

    out["outWT"] = np.ascontiguousarray(
        inputs["out_proj_w"][:, own].T).astype(np.float16)  # [128, 256]

    xprojP = np.zeros((128, 384), np.float32)
    xpf = inputs["xproj_w_f"][:, perm].T  # [512, 48]
    xpb = inputs["xproj_w_b"][:, perm].T
    for kt in range(4):
        xprojP[:, kt * 96:kt * 96 + 48] = xpf[kt * 128:(kt + 1) * 128]
        xprojP[:, kt * 96 + 48:kt * 96 + 96] = xpb[kt * 128:(kt + 1) * 128]
    out["xprojP"] = xprojP.astype(np.float16)

    out["dtprojp"] = np.ascontiguousarray(np.concatenate(
        [inputs["dtproj_w_f"][own].T, inputs["dtproj_w_b"][own].T],
        axis=1)).astype(np.float16)  # [16, 256]

    wconst = np.zeros((128, 76), np.float32)
    for i, d in enumerate("fb"):
        wconst[:, 16 * i:16 * i + 16] = -np.exp(
            inputs[f"A_log_{d}"][own].astype(np.float64))
        cb = inputs[f"conv_b_{d}"][perm]
        wconst[:, 64 + 4 * i:68 + 4 * i] = cb.reshape(4, 128).T
        wconst[:, 72 + i] = inputs[f"D_{d}"][own]
        wconst[:, 74 + i] = inputs[f"dtproj_b_{d}"][own]
    out["wconst"] = wconst
    return out


_CACHE = {}


def kernel(**inputs):
    inputs = {k: np.asarray(v) for k, v in inputs.items()}
    if "nc" not in _CACHE:
        _CACHE["nc"] = build_nc()
    nc = _CACHE["nc"]

    core_ids = list(range(8))
    in_maps = [prep_core_inputs(inputs, core // 4, core % 4)
               for core in core_ids]
    import os
    trace = os.environ.get("BASS_KERNEL_TRACE", "0") == "1"
    res = run_bass_kernel_spmd(nc, in_maps, core_ids, trace=trace)
    _CACHE["last_results"] = res

    x = inputs["x"].astype(np.float32)
    out = np.empty((B, L, 256), np.float32)
    for b in range(B):
        acc = np.zeros((256, L), np.float32)
        for dq in range(4):
            acc += res.results[4 * b + dq]["attnT"]
        out[b] = x[b] + acc.T
    return out.astype(np.float32)


# revision 13
# speedup vs baseline: 1.0146x; 1.0146x over previous
"""Bidirectional Mamba block (nn_Block_bi_mamba) Trainium2 Bass kernel.

Sharding: 8 cores = (batch b in {0,1}) x (d_inner quarter dq in {0..3}).
Each core computes, for its batch and both scan directions, the full
in_proj+conv (folded into PE matmuls) and x_proj (contracts over all 512
channels), the selective scan for its own 128 channels, and the out_proj
partial product [256, L]. The host sums the 4 partials per batch and
adds the residual x. The d_inner axis is permuted per core so the core's
own channel block is always channel-tile 0, making the device program
identical across cores (SPMD) with only input data differing.

Device layout: d-major [128 chans, time]. The causal depthwise conv is
folded into the in_proj matmul (8 shifted PSUM-accumulated matmuls).
The SSM recurrence runs on the DVE tensor_tensor_scan instruction
(state = dA*state + dBu along the free dim); the scan cluster is all
fp16 (2-byte operands run the scan at ~2x the fp32 rate; fp16 keeps the
near-1.0 resolution of dA that bf16 lacks). B/C rows are broadcast
across partitions via DRAM-source stride-0 DMA. The per-(dir,chunk)
front-end (conv matmuls -> silu -> x_proj -> dt) is software-pipelined
one step ahead of the scan phase to hide its serial latency.

Self-contained: hardcodes all shapes; no sibling imports.
"""
import numpy as np
import ml_dtypes
from contextlib import ExitStack

import concourse.bacc as bacc
import concourse.bass as bass
import concourse.tile as tile
from concourse import mybir
from concourse.bass_utils import run_bass_kernel_spmd
from concourse.alu_op_type import AluOpType as CCE

bf = ml_dtypes.bfloat16
FP32 = mybir.dt.float32
BF16 = mybir.dt.bfloat16
FP16 = mybir.dt.float16

B, L = 2, 4096
LC = 2048
NCH = L // LC
NSUB = LC // 512
N = 16
AOP = mybir.AluOpType
AF = mybir.ActivationFunctionType


def _bcast_from_dram(nc, dst, row):
    """DMA-broadcast a [1, F] DRAM row across all partitions of dst."""
    rap = [list(x) for x in row.ap]
    src = bass.AP(tensor=row.tensor, offset=row.offset,
                  ap=[[0, dst.shape[0]], rap[1]])
    nc.sync.dma_start(out=dst, in_=src)


def build_program(tc, ins, outs):
    nc = tc.nc
    with ExitStack() as ctx:
        wp = ctx.enter_context(tc.tile_pool(name="wp", bufs=1))
        big = ctx.enter_context(tc.tile_pool(name="big", bufs=1))
        work = ctx.enter_context(tc.tile_pool(name="work", bufs=1))
        scanp = ctx.enter_context(tc.tile_pool(name="scanp", bufs=2))
        ps = ctx.enter_context(tc.tile_pool(name="ps", bufs=2, space="PSUM"))
        dramp = ctx.enter_context(tc.tile_pool(name="dramp", bufs=3,
                                               space="DRAM"))

        # ---- weights ----
        # wconst fp32 [128, 76]: An_f 0:16 | An_b 16:32 | (unused) |
        #   convb_f 64:68 | convb_b 68:72 | Dd_f 72 | Dd_b 73 |
        #   dtbias_f 74 | dtbias_b 75
        wconst = wp.tile([128, 76], FP32, tag="wconst")
        nc.sync.dma_start(out=wconst, in_=ins["wconst"])
        COL = {"An_f": 0, "An_b": 16, "cb_f": 64, "cb_b": 68, "Dd_f": 72,
               "Dd_b": 73, "dtb_f": 74, "dtb_b": 75}

        wconv = wp.tile([128, 16 * 512], BF16, tag="wconv")
        nc.sync.dma_start(out=wconv, in_=ins["wconvP"])
        wz = wp.tile([128, 256], BF16, tag="wz")
        nc.sync.dma_start(out=wz, in_=ins["wzP"])
        outw = wp.tile([128, 256], FP16, tag="outw")
        nc.sync.dma_start(out=outw, in_=ins["outWT"])
        xpro = wp.tile([128, 384], FP16, tag="xpro")
        nc.sync.dma_start(out=xpro, in_=ins["xprojP"])
        dtprojp = wp.tile([16, 256], FP16, tag="dtprojp")
        nc.sync.dma_start(out=dtprojp, in_=ins["dtprojp"])
        carry = wp.tile([128, 32], FP32, tag="carry")

        # ---- persistent buffers ----
        xtp = [big.tile([128, L + 6], BF16, tag=f"xtp{kt}", name=f"xtp{kt}")
               for kt in range(2)]
        for kt in range(2):
            nc.sync.dma_start(out=xtp[kt],
                              in_=ins["xT16p"][kt * 128:(kt + 1) * 128])
        zs_all = big.tile([128, L], FP16, tag="zs")
        y_ball = big.tile([128, L], FP16, tag="yball")

        # ---- phase Z (emitted as a function; interleaved below) ----
        def phase_z(c):
            cr = slice(c * LC, (c + 1) * LC)
            zsb = work.tile([128, LC], FP16, tag="zsb")
            for nsub in range(NSUB):
                pt = ps.tile([128, 512], FP32, tag="ps_conv")
                for kt in range(2):
                    nc.tensor.matmul(
                        pt, wz[:, kt * 128:(kt + 1) * 128],
                        xtp[kt][:, 3 + c * LC + nsub * 512:
                                3 + c * LC + (nsub + 1) * 512],
                        start=(kt == 0), stop=(kt == 1))
                nc.scalar.copy(out=zsb[:, nsub * 512:(nsub + 1) * 512],
                               in_=pt)
            sgz = work.tile([128, LC], FP16, tag="sgz")
            nc.scalar.activation(out=sgz, in_=zsb, func=AF.Sigmoid,
                                 bias=0.0, scale=1.0)
            nc.vector.tensor_tensor(zs_all[:, cr], zsb, sgz, AOP.mult)

        combos = ([("b", c) for c in range(NCH - 1, -1, -1)]
                  + [("f", c) for c in range(NCH)])

        def front_end(d, c):
            """conv+silu -> xc; x_proj -> dbl + DRAM scratch; dt."""
            dcol = 0 if d == "f" else 1
            base = 0 if d == "f" else 3
            cb0 = COL[f"cb_{d}"]
            xc = [work.tile([128, LC], FP16, tag=f"xc{t}", name=f"xc{t}",
                            bufs=(2 if t == 0 else 1)) for t in range(4)]
            for mt in range(4):
                vcv = work.tile([128, LC], FP16, tag="vcv", bufs=2)
                for nsub in range(NSUB):
                    pt = ps.tile([128, 512], FP32, tag="ps_conv")
                    ns0 = c * LC + nsub * 512 + base
                    for idx, (k, kt) in enumerate(
                            (k, kt) for k in range(4) for kt in range(2)):
                        seg = ((dcol * 4 + k) * 2 + kt) * 512
                        nc.tensor.matmul(
                            pt, wconv[:, seg + mt * 128:seg + (mt + 1) * 128],
                            xtp[kt][:, ns0 + k:ns0 + k + 512],
                            start=(idx == 0), stop=(idx == 7))
                    # v = psum + conv_b via table-free Identity(+bias)
                    nc.scalar.activation(
                        out=vcv[:, nsub * 512:(nsub + 1) * 512], in_=pt,
                        func=AF.Identity,
                        bias=wconst[:, cb0 + mt:cb0 + mt + 1], scale=1.0)
                sgb = work.tile([128, LC], FP16, tag="sgb", bufs=2)
                nc.scalar.activation(out=sgb, in_=vcv, func=AF.Sigmoid,
                                     bias=0.0, scale=1.0)
                nc.vector.tensor_tensor(xc[mt], vcv, sgb, AOP.mult)

            dbl = work.tile([48, LC], FP16, tag="dbl", bufs=2)
            for nsub in range(NSUB):
                pj = ps.tile([48, 512], FP32, tag="ps_xp")
                for kt in range(4):
                    nc.tensor.matmul(
                        pj, xpro[:, kt * 96 + 48 * dcol:
                                 kt * 96 + 48 * (dcol + 1)],
                        xc[kt][:, nsub * 512:(nsub + 1) * 512],
                        start=(kt == 0), stop=(kt == 3))
                nc.scalar.copy(out=dbl[:, nsub * 512:(nsub + 1) * 512],
                               in_=pj)
            scratch = dramp.tile([32, LC], FP16, tag="bcdram")
            nc.sync.dma_start(out=scratch, in_=dbl[16:48, :])

            dt = work.tile([128, LC], FP16, tag="dt", bufs=2)
            for nsub in range(NSUB):
                pt = ps.tile([128, 512], FP32, tag="ps_dt")
                nc.tensor.matmul(
                    pt, dtprojp[:, dcol * 128:(dcol + 1) * 128],
                    dbl[0:16, nsub * 512:(nsub + 1) * 512],
                    start=True, stop=True)
                esub = work.tile([128, 512], FP16, tag="esub", bufs=2)
                nc.scalar.activation(
                    out=esub, in_=pt, func=AF.Exp,
                    bias=wconst[:, COL[f"dtb_{d}"]:COL[f"dtb_{d}"] + 1],
                    scale=1.0)
                nc.scalar.activation(
                    out=dt[:, nsub * 512:(nsub + 1) * 512], in_=esub,
                    func=AF.Ln, bias=1.0, scale=1.0)

            du = work.tile([128, LC], FP16, tag="du", bufs=2)
            nc.vector.tensor_tensor(du, dt, xc[0], AOP.mult)
            return {"xc0": xc[0], "dt": dt, "du": du, "scratch": scratch}

        def scan_phase(d, c, st, first):
            dcol = 0 if d == "f" else 1
            rev = (lambda ap: ap[:, ::-1]) if d == "b" else (lambda ap: ap)
            cr = slice(c * LC, (c + 1) * LC)
            dt, du, scratch, u = st["dt"], st["du"], st["scratch"], st["xc0"]

            yacc = scanp.tile([128, LC], FP16, tag="yacc", bufs=2)
            for n in range(N):
                dA = scanp.tile([128, LC], FP16, tag="dA", bufs=3)
                nc.scalar.activation(
                    out=dA, in_=dt, func=AF.Exp, bias=0.0,
                    scale=wconst[:, COL[f"An_{d}"] + n:COL[f"An_{d}"] + n + 1])
                bbc = scanp.tile([128, LC], FP16, tag="bbc", bufs=3)
                _bcast_from_dram(nc, bbc, scratch[n:n + 1, :])
                nc.vector.tensor_tensor(bbc, du, bbc, AOP.mult)
                h = scanp.tile([128, LC], FP16, tag="h", bufs=3)
                init = (0.0 if first
                        else carry[:, dcol * 16 + n:dcol * 16 + n + 1])
                nc.vector.tensor_tensor_scan(h, rev(dA), rev(bbc), init,
                                             AOP.mult, AOP.add)
                if first and NCH > 1:
                    nc.vector.tensor_copy(
                        out=carry[:, dcol * 16 + n:dcol * 16 + n + 1],
                        in_=h[:, LC - 1:LC])
                cbc = scanp.tile([128, LC], FP16, tag="cbc", bufs=3)
                _bcast_from_dram(nc, cbc, scratch[16 + n:17 + n, :])
                nc.vector.tensor_tensor(h, h, rev(cbc), AOP.mult)
                if n == 0:
                    nc.vector.tensor_copy(out=yacc, in_=h)
                else:
                    nc.vector.tensor_tensor(yacc, yacc, h, AOP.add)

            yfin = yacc
            Dsl = wconst[:, COL[f"Dd_{d}"]:COL[f"Dd_{d}"] + 1]

            if d == "b":
                nc.vector.scalar_tensor_tensor(
                    y_ball[:, cr], u, Dsl, yfin[:, ::-1], AOP.mult, AOP.add)
            else:
                yg = work.tile([128, LC], FP16, tag="ytmp", bufs=2)
                nc.vector.scalar_tensor_tensor(
                    yg, u, Dsl, yfin, AOP.mult, AOP.add)
                ysum = work.tile([128, LC], FP16, tag="ytmp", bufs=2)
                nc.vector.tensor_tensor(ysum, yg, y_ball[:, cr], AOP.add)
                ygated = work.tile([128, LC], FP16, tag="ytmp", bufs=2)
                nc.vector.tensor_tensor(ygated, ysum, zs_all[:, cr], AOP.mult)
                for mt in range(2):
                    osb = work.tile([128, LC], FP32, tag="osb")
                    for nsub in range(NSUB):
                        po = ps.tile([128, 512], FP32, tag="ps_out")
                        nc.tensor.matmul(
                            po, outw[:, mt * 128:(mt + 1) * 128],
                            ygated[:, nsub * 512:(nsub + 1) * 512],
                            start=True, stop=True)
                        nc.scalar.copy(
                            out=osb[:, nsub * 512:(nsub + 1) * 512], in_=po)
                    nc.sync.dma_start(
                        out=outs["attnT"][mt * 128:(mt + 1) * 128, cr],
                        in_=osb)

        # software pipeline: front_end one combo ahead of the scan phase;
        # phase-Z rides in the shadow of the first front-end
        states = {}
        states[0] = front_end(*combos[0])
        for c in range(NCH):
            phase_z(c)
        for j, (d, c) in enumerate(combos):
            if j + 1 < len(combos):
                states[j + 1] = front_end(*combos[j + 1])
            first = (j % NCH == 0)
            scan_phase(d, c, states.pop(j), first)


def build_nc():
    nc = bacc.Bacc("TRN2", target_bir_lowering=False, debug=False,
                   enable_asserts=False)
    ins = {}

    def inp(name, shape, dt):
        ins[name] = nc.dram_tensor(name, shape, dt,
                                   kind="ExternalInput").ap()

    inp("xT16p", [256, L + 6], BF16)
    inp("wconvP", [128, 16 * 512], BF16)
    inp("wzP", [128, 256], BF16)
    inp("outWT", [128, 256], FP16)
    inp("xprojP", [128, 384], FP16)
    inp("dtprojp", [16, 256], FP16)
    inp("wconst", [128, 76], FP32)
    outs = {"attnT": nc.dram_tensor("attnT", [256, L], FP32,
                                    kind="ExternalOutput").ap()}
    with tile.TileContext(nc) as tc:
        build_program(tc, ins, outs)
    nc.compile()
    return nc


def prep_core_inputs(inputs, b, dq):
    """Per-core input arrays; d_inner axis permuted so own block is first."""
    own = np.arange(dq * 128, (dq + 1) * 128)
    rest = np.array([i for i in range(512)
                     if not (dq * 128 <= i < (dq + 1) * 128)])
    perm = np.concatenate([own, rest])

    out = {}
    xT = inputs["x"][b].T.astype(np.float32)  # [256, L]
    xTp = np.zeros((256, L + 6), np.float32)
    xTp[:, 3:L + 3] = xT
    out["xT16p"] = xTp.astype(bf)

    w_inx = inputs["in_proj_w"][:512][perm].astype(np.float32)  # [512, 256]
    wconvP = np.zeros((128, 16 * 512), np.float32)
    for dcol, d in enumerate("fb"):
        cw = inputs[f"conv_w_{d}"][:, 0, :][perm].astype(np.float32)
        for k in range(4):
            tap = cw[:, k] if d == "f" else cw[:, 3 - k]
            WdkT = (tap[:, None] * w_inx).T     # [256, 512]
            for kt in range(2):
                seg = ((dcol * 4 + k) * 2 + kt) * 512
                wconvP[:, seg:seg + 512] = WdkT[kt * 128:(kt + 1) * 128]
    out["wconvP"] = wconvP.astype(bf)

    wz = inputs["in_proj_w"][512:1024][own].astype(np.float32)  # [128, 256]
    wzP = np.zeros((128, 256), np.float32)
    for kt in range(2):
        wzP[:, kt * 128:(kt + 1) * 128] = wz.T[kt * 128:(kt + 1) * 128]
    out["wzP"] = wzP.astype(bf)

    out["outWT"] = np.ascontiguousarray(
        inputs["out_proj_w"][:, own].T).astype(np.float16)  # [128, 256]

    xprojP = np.zeros((128, 384), np.float32)
    xpf = inputs["xproj_w_f"][:, perm].T  # [512, 48]
    xpb = inputs["xproj_w_b"][:, perm].T
    for kt in range(4):
        xprojP[:, kt * 96:kt * 96 + 48] = xpf[kt * 128:(kt + 1) * 128]
        xprojP[:, kt * 96 + 48:kt * 96 + 96] = xpb[kt * 128:(kt + 1) * 128]
    out["xprojP"] = xprojP.astype(np.float16)

    out["dtprojp"] = np.ascontiguousarray(np.concatenate(
        [inputs["dtproj_w_f"][own].T, inputs["dtproj_w_b"][own].T],
        axis=1)).astype(np.float16)  # [16, 256]

    wconst = np.zeros((128, 76), np.float32)
    for i, d in enumerate("fb"):
        wconst[:, 16 * i:16 * i + 16] = -np.exp(
            inputs[f"A_log_{d}"][own].astype(np.float64))
        cb = inputs[f"conv_b_{d}"][perm]
        wconst[:, 64 + 4 * i:68 + 4 * i] = cb.reshape(4, 128).T
        wconst[:, 72 + i] = inputs[f"D_{d}"][own]
        wconst[:, 74 + i] = inputs[f"dtproj_b_{d}"][own]
    out["wconst"] = wconst
    return out


_CACHE = {}


def kernel(**inputs):
    inputs = {k: np.asarray(v) for k, v in inputs.items()}
    if "nc" not in _CACHE:
        _CACHE["nc"] = build_nc()
    nc = _CACHE["nc"]

    core_ids = list(range(8))
    in_maps = [prep_core_inputs(inputs, core // 4, core % 4)
               for core in core_ids]
    import os
    trace = os.environ.get("BASS_KERNEL_TRACE", "0") == "1"
    res = run_bass_kernel_spmd(nc, in_maps, core_ids, trace=trace)
    _CACHE["last_results"] = res

    x = inputs["x"].astype(np.float32)
    out = np.empty((B, L, 256), np.float32)
    for b in range(B):
        acc = np.zeros((256, L), np.float32)
        for dq in range(4):
            acc += res.results[4 * b + dq]["attnT"]
        out[b] = x[b] + acc.T
    return out.astype(np.float32)
